# revision 1
# baseline (speedup 1.0000x reference)
"""CascadedGroupAttention Trainium2 kernel (v2, bf16).

Data-parallel over batch: B=512 -> 64 samples x 8 cores. Head-outer loop
(all 64 samples per head, then next head) so the Gelu/Exp ACT-table
reloads happen twice per head instead of twice per head-block.

Structure per head:
  phase A: k/q matmuls (col-tiled, 4 samples/bank, bf16), depthwise 5x5
    conv as 25 PSUM-accumulated diagonal matmuls over 8-sample free-packed
    padded tiles, exact Gelu on ACT (+residual on DVE), v^T computed
    directly as feat^T @ Wv matmuls (64-col), softmax ones-column folded
    into the Wv matrix via feat's ones row.
  phase B (per sample pair): QK^T into a 2-bank PSUM tile (m-tiles 128+68),
    rel-pos bias added via identity matmuls (bf16), one Exp ACT per pair
    (bf16 out), AV with denominator row, gpsimd broadcast + DVE
    reciprocal/scale/relu tail feeding the cascade and the relu'd concat.
  projection: per block, bf16 weights, fp32 out.

All matmul operands bf16 (PSUM accumulation fp32); x is shipped bf16 with
a ones row packed per head chunk on host.
"""

import numpy as np
import sys

sys.path.insert(0, "/opt/trn_rl_repo")

import ml_dtypes  # noqa: E402

import concourse.bass as bass  # noqa: E402
from concourse import bacc  # noqa: E402
import concourse.mybir as mybir  # noqa: E402
from concourse.tile import TileContext  # noqa: E402

F32 = mybir.dt.float32
BF16 = mybir.dt.bfloat16
BF = ml_dtypes.bfloat16

NHEADS = 4
KD = 16          # key dim
DV = 64          # per-head value dim
CH = 64          # per-head input channels
RES = 14
N = RES * RES    # 196 tokens
DIM = 256
BATCH = 512
NCORES = 8
SPC = BATCH // NCORES   # 64 samples per core
SCALE = KD ** -0.5

TAPS = [(0, 0)] + [
    (dr, dc) for dr in range(-2, 3) for dc in range(-2, 3) if (dr, dc) != (0, 0)
]


def _prep_host(inp):
    """Fold BN affines into weights; build bf16 hardware-layout arrays."""
    qkv_w = np.asarray(inp["qkv_w"], np.float32)
    qkv_scale = np.asarray(inp["qkv_scale"], np.float32)
    qkv_bias = np.asarray(inp["qkv_bias"], np.float32)
    dw_w = np.asarray(inp["dw_w"], np.float32)
    dw_scale = np.asarray(inp["dw_scale"], np.float32)
    dw_bias = np.asarray(inp["dw_bias"], np.float32)
    proj_w = np.asarray(inp["proj_w"], np.float32)
    proj_scale = np.asarray(inp["proj_scale"], np.float32)
    proj_bias = np.asarray(inp["proj_bias"], np.float32)
    ab_full = np.asarray(inp["attention_biases"], np.float32)[
        :, np.asarray(inp["bias_idxs"])
    ]  # [4, 196, 196], symmetric

    w_k = np.zeros((NHEADS, CH + 1, 32), np.float32)
    w_q = np.zeros((NHEADS, CH + 1, 32), np.float32)
    w_v = np.zeros((NHEADS, CH + 1, DV + 1), np.float32)
    conv_diag = np.zeros((NHEADS, 128, 25, 128), np.float32)
    dwb_pat = np.zeros((NHEADS, 128, 1), np.float32)
    for i in range(NHEADS):
        for j in range(KD):
            w_k[i, :CH, j] = qkv_w[i, KD + j] * qkv_scale[i, KD + j] * SCALE
            w_k[i, CH, j] = qkv_bias[i, KD + j] * SCALE
            w_q[i, :CH, j] = qkv_w[i, j] * qkv_scale[i, j]
            w_q[i, CH, j] = qkv_bias[i, j]
        for d in range(DV):
            w_v[i, :CH, d] = qkv_w[i, 2 * KD + d] * qkv_scale[i, 2 * KD + d]
            w_v[i, CH, d] = qkv_bias[i, 2 * KD + d]
        w_v[i, CH, DV] = 1.0   # ones column -> softmax denominator row in AV
        for t, (dr, dc) in enumerate(TAPS):
            for p in range(128):
                c = p % 32
                if c < KD:
                    conv_diag[i, p, t, p] = (
                        dw_w[i, c, dr + 2, dc + 2] * dw_scale[i, c]
                    )
        for p in range(128):
            c = p % 32
            if c < KD:
                dwb_pat[i, p, 0] = dw_bias[i, c]

    ident = np.eye(128, dtype=np.float32)
    proj_wT = np.ascontiguousarray((proj_w * proj_scale[:, None]).T)
    pb = np.ascontiguousarray(proj_bias.reshape(2, 128, 1).astype(np.float32))

    return {
        "w_k": w_k.astype(BF),
        "w_q": w_q.astype(BF),
        "w_v": w_v.astype(BF),
        "conv_diag": conv_diag.astype(BF),
        "dwb_pat": dwb_pat,
        "ab": np.ascontiguousarray(ab_full).astype(BF),
        "ident": ident.astype(BF),
        "proj_wT": proj_wT.astype(BF),
        "proj_b": pb,
    }


def _pack_x(x):
    """[B, 256, N] f32 -> [B, 4, 65, N] bf16 with a ones row per head chunk."""
    B = x.shape[0]
    xp = np.empty((B, NHEADS, CH + 1, N), dtype=BF)
    xr = x.reshape(B, NHEADS, CH, N)
    xp[:, :, :CH, :] = xr.astype(BF)
    xp[:, :, CH, :] = np.ones((1,), dtype=BF)
    return xp


def build_bass(spc=SPC):
    nc = bacc.Bacc(None, target_bir_lowering=False)

    x_d = nc.declare_dram_parameter("x", [spc, NHEADS, CH + 1, N], BF16,
                                    isOutput=False)
    wk_d = nc.declare_dram_parameter("w_k", [NHEADS, CH + 1, 32], BF16,
                                     isOutput=False)
    wq_d = nc.declare_dram_parameter("w_q", [NHEADS, CH + 1, 32], BF16,
                                     isOutput=False)
    wv_d = nc.declare_dram_parameter("w_v", [NHEADS, CH + 1, DV + 1], BF16,
                                     isOutput=False)
    cdiag_d = nc.declare_dram_parameter("conv_diag", [NHEADS, 128, 25, 128],
                                        BF16, isOutput=False)
    dwb_d = nc.declare_dram_parameter("dwb_pat", [NHEADS, 128, 1], F32,
                                      isOutput=False)
    ab_d = nc.declare_dram_parameter("ab", [NHEADS, N, N], BF16, isOutput=False)
    id_d = nc.declare_dram_parameter("ident", [128, 128], BF16, isOutput=False)
    pw_d = nc.declare_dram_parameter("proj_wT", [DIM, DIM], BF16, isOutput=False)
    pb_d = nc.declare_dram_parameter("proj_b", [2, 128, 1], F32, isOutput=False)
    out_d = nc.declare_dram_parameter("out", [spc, DIM, N], F32, isOutput=True)

    nhb = spc // 8          # half-blocks of 8 samples
    npair = spc // 2

    with TileContext(nc) as tc:
        with (
            tc.tile_pool(name="const", bufs=1) as constp,
            tc.tile_pool(name="persist", bufs=1) as persist,
            tc.tile_pool(name="work", bufs=3) as work,
            tc.tile_pool(name="outp", bufs=4) as outp,
            tc.tile_pool(name="psA", bufs=3, space="PSUM") as psA,
            tc.tile_pool(name="psT", bufs=2, space="PSUM") as psT,
            tc.tile_pool(name="psV", bufs=1, space="PSUM") as psV,
        ):
            # ---- constants ----
            ident = constp.tile([128, 128], BF16, name="ident")
            nc.sync.dma_start(out=ident, in_=id_d[:, :])
            wk_sb, wq_sb, wv_sb, dwb_sb, ab0_sb, ab1_sb = [], [], [], [], [], []
            for i in range(NHEADS):
                t = constp.tile([CH + 1, 32], BF16, name=f"wk{i}")
                nc.sync.dma_start(out=t, in_=wk_d[i])
                wk_sb.append(t)
                t = constp.tile([CH + 1, 32], BF16, name=f"wq{i}")
                nc.sync.dma_start(out=t, in_=wq_d[i])
                wq_sb.append(t)
                t = constp.tile([CH + 1, DV + 1], BF16, name=f"wv{i}")
                nc.sync.dma_start(out=t, in_=wv_d[i])
                wv_sb.append(t)
                t = constp.tile([128, 1], F32, name=f"dwb{i}")
                nc.sync.dma_start(out=t, in_=dwb_d[i])
                dwb_sb.append(t)
                t = constp.tile([128, N], BF16, name=f"ab0_{i}")
                nc.sync.dma_start(out=t, in_=ab_d[i, 0:128, :])
                ab0_sb.append(t)
                t = constp.tile([68, N], BF16, name=f"ab1_{i}")
                nc.sync.dma_start(out=t, in_=ab_d[i, 128:196, :])
                ab1_sb.append(t)
            pw0 = constp.tile([128, DIM], BF16, name="pw0")
            nc.sync.dma_start(out=pw0, in_=pw_d[0:128, :])
            pw1 = constp.tile([128, DIM], BF16, name="pw1")
            nc.sync.dma_start(out=pw1, in_=pw_d[128:256, :])
            pb0 = constp.tile([128, 1], F32, name="pb0")
            nc.sync.dma_start(out=pb0, in_=pb_d[0])
            pb1 = constp.tile([128, 1], F32, name="pb1")
            nc.sync.dma_start(out=pb1, in_=pb_d[1])

            # ---- persistent state ----
            feat = []
            for sl in range(spc):
                t = persist.tile([CH + 1, N], BF16, name=f"feat{sl}",
                                 tag=f"feat{sl}")
                nc.sync.dma_start(out=t, in_=x_d[sl, 0])
                feat.append(t)
            rcA, rcB = [], []
            for p in range(npair):
                rcA.append(persist.tile([128, 392], BF16, name=f"rA{p}",
                                        tag=f"rA{p}"))
                rcB.append(persist.tile([128, 392], BF16, name=f"rB{p}",
                                        tag=f"rB{p}"))
            # padded conv inputs: [4s*32r, 2 banks, 18, 18]; halo zeroed once
            qpad = []
            for hb in range(nhb):
                t = persist.tile([128, 2 * 18 * 18], BF16, name=f"qpad{hb}",
                                 tag=f"qpad{hb}")
                nc.vector.memset(t, 0.0)
                qpad.append(t)

            for i in range(NHEADS):
                cdg = work.tile([128, 25, 128], BF16, name=f"cdg{i}",
                                tag="cdg", bufs=2)
                nc.sync.dma_start(out=cdg, in_=cdiag_d[i])
                xn = None
                if i < NHEADS - 1:
                    xn = []
                    for sl in range(spc):
                        t = work.tile([CH, N], BF16, name=f"xn{i}_{sl}",
                                      tag=f"xn{sl}", bufs=1)
                        nc.sync.dma_start(out=t, in_=x_d[sl, i + 1, 0:CH, :])
                        xn.append(t)

                # ================= phase A =================
                kf = [None] * (spc // 4)
                qf = [None] * nhb
                vts = [None] * npair
                for hb in range(nhb):
                    qp3 = qpad[hb].rearrange("p (g r c) -> p g r c", g=2, c=18)
                    for gg in range(2):
                        g = 2 * hb + gg
                        kqp = psA.tile([128, 512], F32, name=f"kqp{g}",
                                       tag="A")
                        for j in range(4):
                            nc.tensor.matmul(
                                kqp[32 * j:32 * j + 32, 0:N],
                                wk_sb[i], feat[4 * g + j],
                                start=True, stop=True,
                                tile_position=(0, 32 * j),
                            )
                            nc.tensor.matmul(
                                kqp[32 * j:32 * j + 32, 196:196 + N],
                                wq_sb[i], feat[4 * g + j],
                                start=True, stop=True,
                                tile_position=(0, 32 * j),
                            )
                        t = persist.tile([128, N], BF16, name=f"kf{i}_{g}",
                                         tag=f"kf{g}")
                        nc.vector.tensor_copy(t, kqp[:, 0:N])
                        kf[g] = t
                        nc.vector.tensor_copy(
                            qp3[:, gg, 2:16, 2:16],
                            kqp[:, 196:196 + N].rearrange(
                                "p (r c) -> p r c", c=RES),
                        )
                    # conv: 25 accumulated diag matmuls over both banks
                    dq = psA.tile([128, 512], F32, name=f"dq{hb}", tag="A")
                    for t_i, (dr, dc) in enumerate(TAPS):
                        nc.tensor.matmul(
                            dq[:, 0:392],
                            cdg[:, t_i, :],
                            qp3[:, :, 2 + dr:16 + dr, 2 + dc:16 + dc],
                            start=(t_i == 0), stop=(t_i == len(TAPS) - 1),
                        )
                    g8 = work.tile([128, 392], BF16, name=f"g8{hb}", tag="g8",
                                   bufs=2)
                    nc.scalar.activation(
                        g8, dq[:, 0:392], mybir.ActivationFunctionType.Gelu,
                        bias=dwb_sb[i], scale=1.0,
                    )
                    t = persist.tile([128, 392], BF16, name=f"qf{i}_{hb}",
                                     tag=f"qf{hb}")
                    nc.vector.tensor_add(
                        t.rearrange("p (g r c) -> p g r c", g=2, c=RES),
                        g8.rearrange("p (g r c) -> p g r c", g=2, c=RES),
                        qp3[:, :, 2:16, 2:16],
                    )
                    qf[hb] = t
                    # v^T: 2 samples per psum tile, both m-tiles
                    for gg in range(2):
                        g = 2 * hb + gg
                        for pp in range(2):
                            p = 2 * g + pp
                            vt = psA.tile([128, 512], F32, name=f"vt{p}",
                                          tag="A")
                            for u2 in range(2):
                                sl = 2 * p + u2
                                nc.tensor.matmul(
                                    vt[0:128, 65 * u2:65 * u2 + 65],
                                    feat[sl][:, 0:128], wv_sb[i],
                                    start=True, stop=True,
                                )
                                nc.tensor.matmul(
                                    vt[0:68, 260 + 65 * u2:260 + 65 * u2 + 65],
                                    feat[sl][:, 128:196], wv_sb[i],
                                    start=True, stop=True,
                                )
                            t = persist.tile([128, 390], BF16,
                                             name=f"vts{i}_{p}",
                                             tag=f"vts{p % 16}")
                            nc.vector.tensor_copy(t[:, 0:130], vt[:, 0:130])
                            nc.vector.tensor_copy(t[0:68, 260:390],
                                                  vt[0:68, 260:390])
                            vts[p] = t

                # ================= phase B =================
                for p in range(npair):
                    g = p // 2
                    pT = psT.tile([128, 1024], F32, name=f"pT{p}", tag="pT")
                    for u2 in range(2):
                        sl = 2 * p + u2
                        j = sl % 4
                        qs = qf[g // 2].rearrange(
                            "p (g r) -> p g r", g=2)[32 * j:32 * j + 16,
                                                     g % 2, :]
                        # one accumulation group per bank at a time; the ab
                        # matmuls (full 128 partitions) open each group so
                        # the group region covers the K=68 QK tile's rows
                        nc.tensor.matmul(
                            pT[0:128, 196 * u2:196 * u2 + 196],
                            ident, ab0_sb[i],
                            start=True, stop=False,
                        )
                        # K=68 with M=128: rows 68:128 get written zeros
                        nc.tensor.matmul(
                            pT[0:128, 512 + 196 * u2:512 + 196 * u2 + 196],
                            ident[0:68, 0:128], ab1_sb[i],
                            start=True, stop=False,
                        )
                        nc.tensor.matmul(
                            pT[0:128, 196 * u2:196 * u2 + 196],
                            kf[g][32 * j:32 * j + 16, 0:128], qs,
                            start=False, stop=True,
                            tile_position=(32 * j, 0),
                        )
                        nc.tensor.matmul(
                            pT[0:68, 512 + 196 * u2:512 + 196 * u2 + 196],
                            kf[g][32 * j:32 * j + 16, 128:196], qs,
                            start=False, stop=True,
                            tile_position=(32 * j, 0),
                        )
                    eP = work.tile([128, 784], BF16, name=f"eP{p}", tag="eP",
                                   bufs=3)
                    nc.scalar.activation(
                        eP.rearrange("p (t n) -> p t n", t=2),
                        pT.rearrange("p (t n) -> p t n", t=2)[:, :, 0:392],
                        mybir.ActivationFunctionType.Exp,
                    )
                    av = psV.tile([65, 512], F32, name=f"av{p}", tag="av")
                    for u2 in range(2):
                        nc.tensor.matmul(
                            av[:, 196 * u2:196 * u2 + 196],
                            vts[p][0:128, 65 * u2:65 * u2 + 65],
                            eP[0:128, 196 * u2:196 * u2 + 196],
                            start=True, stop=False,
                        )
                        nc.tensor.matmul(
                            av[:, 196 * u2:196 * u2 + 196],
                            vts[p][0:68, 260 + 65 * u2:260 + 65 * u2 + 65],
                            eP[0:68, 392 + 196 * u2:392 + 196 * u2 + 196],
                            start=False, stop=True,
                        )
                    dsb = work.tile([1, 392], F32, name="dsb", tag="dsb",
                                    bufs=3)
                    nc.scalar.copy(dsb, av[64:65, 0:392])
                    rcp = work.tile([1, 392], F32, name="rcp", tag="rcp",
                                    bufs=3)
                    nc.vector.reciprocal_approx_fast(rcp, dsb)
                    bc = work.tile([64, 392], F32, name="bc", tag="bc",
                                   bufs=3)
                    nc.gpsimd.partition_broadcast(bc, rcp[0:1, :])
                    for u2 in range(2):
                        sl = 2 * p + u2
                        rc = (rcA if i < 2 else rcB)[p][
                            64 * (i % 2):64 * (i % 2) + 64,
                            196 * u2:196 * u2 + 196,
                        ]
                        avs = av[0:64, 196 * u2:196 * u2 + 196]
                        bcs = bc[:, 196 * u2:196 * u2 + 196]
                        if i < NHEADS - 1:
                            avdiv = work.tile([64, N], BF16, name="avdiv",
                                              tag="avdiv", bufs=4)
                            nc.vector.tensor_mul(avdiv, avs, bcs)
                            nc.vector.tensor_scalar_max(rc, avdiv, 0.0)
                            nc.vector.tensor_add(
                                feat[sl][0:CH, :], avdiv, xn[sl])
                        else:
                            nc.vector.scalar_tensor_tensor(
                                rc, avs, 0.0, bcs,
                                op0=mybir.AluOpType.max,
                                op1=mybir.AluOpType.mult,
                            )
                    # ---- projection (head 3, after each block's 8 pairs) ----
                    if i == NHEADS - 1 and p % 8 == 7:
                        for pp in range(p - 7, p + 1):
                            for m in range(2):
                                op = psA.tile([128, 512], F32,
                                              name=f"op{pp}_{m}", tag="A")
                                nc.tensor.matmul(
                                    op[:, 0:392],
                                    pw0[:, 128 * m:128 * m + 128], rcA[pp],
                                    start=True, stop=False,
                                )
                                nc.tensor.matmul(
                                    op[:, 0:392],
                                    pw1[:, 128 * m:128 * m + 128], rcB[pp],
                                    start=False, stop=True,
                                )
                                ob = outp.tile([128, 392], F32,
                                               name=f"ob{m}", tag=f"ob{m}")
                                if m == 0:
                                    nc.scalar.activation(
                                        ob, op[:, 0:392],
                                        mybir.ActivationFunctionType.Identity,
                                        bias=pb0, scale=1.0,
                                    )
                                else:
                                    nc.vector.tensor_scalar_add(
                                        ob, op[:, 0:392], pb1)
                                nc.sync.dma_start(
                                    out=out_d[
                                        2 * pp:2 * pp + 2,
                                        128 * m:128 * m + 128,
                                        :,
                                    ].rearrange("s o n -> o s n"),
                                    in_=ob.rearrange("o (s n) -> o s n", s=2),
                                )
    nc.finalize()
    return nc


_CACHE = {}


def _get_nc():
    if "nc" not in _CACHE:
        _CACHE["nc"] = build_bass()
    return _CACHE["nc"]


def build_inmaps(inputs):
    host = _prep_host(inputs)
    xp = _pack_x(np.asarray(inputs["x"], np.float32).reshape(BATCH, DIM, N))
    in_maps = []
    for c in range(NCORES):
        m = {"x": np.ascontiguousarray(xp[c * SPC:(c + 1) * SPC])}
        m.update(host)
        in_maps.append(m)
    return in_maps


def kernel(**inputs) -> np.ndarray:
    from concourse.bass_utils import run_bass_kernel_spmd

    nc = _get_nc()
    in_maps = build_inmaps(inputs)
    res = run_bass_kernel_spmd(nc, in_maps, list(range(NCORES)))
    out = np.concatenate([r["out"] for r in res.results], axis=0)
    return out.reshape(BATCH, DIM, RES, RES).astype(np.float32)



# revision 27
# speedup vs baseline: 1.1902x; 1.1902x over previous
"""CascadedGroupAttention Trainium2 kernel (v2 reference reconstruction)."""

import numpy as np
import sys

sys.path.insert(0, "/opt/trn_rl_repo")

import ml_dtypes  # noqa: E402

import concourse.bass as bass  # noqa: E402
from concourse import bacc  # noqa: E402
import concourse.mybir as mybir  # noqa: E402
from concourse.tile import TileContext  # noqa: E402

F32 = mybir.dt.float32
BF16 = mybir.dt.bfloat16
BF = ml_dtypes.bfloat16

NHEADS = 4
KD = 16
DV = 64
CH = 64
RES = 14
N = RES * RES
DIM = 256
BATCH = 512
NCORES = 8
SPC = BATCH // NCORES
SCALE = KD ** -0.5

TAPS = [(0, 0)] + [
    (dr, dc) for dr in range(-2, 3) for dc in range(-2, 3) if (dr, dc) != (0, 0)
]


def _prep_host(inp):
    qkv_w = np.asarray(inp["qkv_w"], np.float32)
    qkv_scale = np.asarray(inp["qkv_scale"], np.float32)
    qkv_bias = np.asarray(inp["qkv_bias"], np.float32)
    dw_w = np.asarray(inp["dw_w"], np.float32)
    dw_scale = np.asarray(inp["dw_scale"], np.float32)
    dw_bias = np.asarray(inp["dw_bias"], np.float32)
    proj_w = np.asarray(inp["proj_w"], np.float32)
    proj_scale = np.asarray(inp["proj_scale"], np.float32)
    proj_bias = np.asarray(inp["proj_bias"], np.float32)
    ab_full = np.asarray(inp["attention_biases"], np.float32)[
        :, np.asarray(inp["bias_idxs"])
    ]

    w_k = np.zeros((NHEADS, CH + 1, 32), np.float32)
    w_q = np.zeros((NHEADS, CH + 1, 32), np.float32)
    w_v = np.zeros((NHEADS, CH + 1, DV + 1), np.float32)
    conv_diag = np.zeros((NHEADS, 128, 25, 128), np.float32)
    dwb_pat = np.zeros((NHEADS, 128, 1), np.float32)
    for i in range(NHEADS):
        for j in range(KD):
            w_k[i, :CH, j] = qkv_w[i, KD + j] * qkv_scale[i, KD + j] * SCALE
            w_k[i, CH, j] = qkv_bias[i, KD + j] * SCALE
            w_q[i, :CH, j] = qkv_w[i, j] * qkv_scale[i, j]
            w_q[i, CH, j] = qkv_bias[i, j]
        for d in range(DV):
            w_v[i, :CH, d] = qkv_w[i, 2 * KD + d] * qkv_scale[i, 2 * KD + d]
            w_v[i, CH, d] = qkv_bias[i, 2 * KD + d]
        w_v[i, CH, DV] = 1.0
        for t, (dr, dc) in enumerate(TAPS):
            for p in range(128):
                c = p % 32
                if c < KD:
                    conv_diag[i, p, t, p] = (
                        dw_w[i, c, dr + 2, dc + 2] * dw_scale[i, c]
                    )
        for p in range(128):
            c = p % 32
            if c < KD:
                dwb_pat[i, p, 0] = dw_bias[i, c]

    ident = np.eye(128, dtype=np.float32)
    proj_wT = np.ascontiguousarray((proj_w * proj_scale[:, None]).T)
    pb = np.ascontiguousarray(proj_bias.reshape(2, 128, 1).astype(np.float32))

    return {
        "w_k": w_k.astype(BF),
        "w_q": w_q.astype(BF),
        "w_v": w_v.astype(BF),
        "conv_diag": conv_diag.astype(BF),
        "dwb_pat": dwb_pat,
        "ab": np.ascontiguousarray(ab_full).astype(BF),
        "ident": ident.astype(BF),
        "proj_wT": proj_wT.astype(BF),
        "proj_b": pb,
    }


def _pack_x(x):
    xr = x.reshape(NCORES, SPC, NHEADS, CH, N)
    xh = np.empty((NCORES, NHEADS, CH + 1, SPC * N), dtype=BF)
    xh[:, :, :CH, :] = (
        xr.transpose(0, 2, 3, 1, 4).reshape(NCORES, NHEADS, CH, SPC * N)
        .astype(BF)
    )
    xh[:, :, CH, :] = np.ones((1,), dtype=BF)
    return xh


def build_bass(spc=SPC):
    nc = bacc.Bacc(None, target_bir_lowering=False)

    xh_d = nc.declare_dram_parameter("xh", [NHEADS, CH + 1, spc * N], BF16,
                                     isOutput=False)
    wk_d = nc.declare_dram_parameter("w_k", [NHEADS, CH + 1, 32], BF16,
                                     isOutput=False)
    wq_d = nc.declare_dram_parameter("w_q", [NHEADS, CH + 1, 32], BF16,
                                     isOutput=False)
    wv_d = nc.declare_dram_parameter("w_v", [NHEADS, CH + 1, DV + 1], BF16,
                                     isOutput=False)
    cdiag_d = nc.declare_dram_parameter("conv_diag", [NHEADS, 128, 25, 128],
                                        BF16, isOutput=False)
    dwb_d = nc.declare_dram_parameter("dwb_pat", [NHEADS, 128, 1], F32,
                                      isOutput=False)
    ab_d = nc.declare_dram_parameter("ab", [NHEADS, N, N], BF16, isOutput=False)
    id_d = nc.declare_dram_parameter("ident", [128, 128], BF16, isOutput=False)
    pw_d = nc.declare_dram_parameter("proj_wT", [DIM, DIM], BF16, isOutput=False)
    pb_d = nc.declare_dram_parameter("proj_b", [2, 128, 1], F32, isOutput=False)
    out_d = nc.declare_dram_parameter("out", [spc, DIM, N], F32, isOutput=True)

    nhb = spc // 8
    npair = spc // 2

    with TileContext(nc) as tc:
        with (
            tc.tile_pool(name="const", bufs=1) as constp,
            tc.tile_pool(name="persist", bufs=1) as persist,
            tc.tile_pool(name="work", bufs=3) as work,
            tc.tile_pool(name="outp", bufs=2) as outp,
            tc.tile_pool(name="psA", bufs=2, space="PSUM") as psA,
            tc.tile_pool(name="psT", bufs=2, space="PSUM") as psT,
            tc.tile_pool(name="psV", bufs=2, space="PSUM") as psV,
        ):
            # ---- constants ----
            ident = constp.tile([128, 128], BF16, name="ident")
            nc.sync.dma_start(out=ident, in_=id_d[:, :])
            wk_sb, wq_sb, wv_sb, dwb_sb, ab0_sb, ab1_sb = [], [], [], [], [], []
            for i in range(NHEADS):
                t = constp.tile([CH + 1, 32], BF16, name=f"wk{i}")
                nc.sync.dma_start(out=t, in_=wk_d[i])
                wk_sb.append(t)
                t = constp.tile([CH + 1, 32], BF16, name=f"wq{i}")
                nc.sync.dma_start(out=t, in_=wq_d[i])
                wq_sb.append(t)
                t = constp.tile([CH + 1, DV + 1], BF16, name=f"wv{i}")
                nc.sync.dma_start(out=t, in_=wv_d[i])
                wv_sb.append(t)
                t = constp.tile([128, 1], F32, name=f"dwb{i}")
                nc.sync.dma_start(out=t, in_=dwb_d[i])
                dwb_sb.append(t)
                t = constp.tile([128, N], BF16, name=f"ab0_{i}")
                nc.sync.dma_start(out=t, in_=ab_d[i, 0:128, :])
                ab0_sb.append(t)
                t = constp.tile([68, N], BF16, name=f"ab1_{i}")
                nc.sync.dma_start(out=t, in_=ab_d[i, 128:196, :])
                ab1_sb.append(t)
            pw0 = constp.tile([128, DIM], BF16, name="pw0")
            nc.sync.dma_start(out=pw0, in_=pw_d[0:128, :])
            pw1 = constp.tile([128, DIM], BF16, name="pw1")
            nc.sync.dma_start(out=pw1, in_=pw_d[128:256, :])
            pb0 = constp.tile([128, 1], F32, name="pb0")
            nc.sync.dma_start(out=pb0, in_=pb_d[0])
            pb1 = constp.tile([128, 1], F32, name="pb1")
            nc.sync.dma_start(out=pb1, in_=pb_d[1])

            # ---- persistent state ----
            fpp = []
            for t_i in range(2):
                t = persist.tile([CH + 1, spc * N], BF16, name=f"fpp{t_i}",
                                 tag=f"fpp{t_i}")
                fpp.append(t)
            nc.sync.dma_start(out=fpp[0], in_=xh_d[0])
            nc.vector.memset(fpp[1][CH:CH + 1, :], 1.0)
            rcA, rcB = [], []
            for p in range(npair):
                rcA.append(persist.tile([128, 392], BF16, name=f"rA{p}",
                                        tag=f"rA{p}"))
                rcB.append(persist.tile([128, 392], BF16, name=f"rB{p}",
                                        tag=f"rB{p}"))
            qpad = []
            for hb in range(nhb):
                t = persist.tile([128, 2 * 18 * 18], BF16, name=f"qpad{hb}",
                                 tag=f"qpad{hb}")
                nc.vector.memset(t, 0.0)
                qpad.append(t)

            for i in range(NHEADS):
                fsrc = fpp[i % 2]
                fdst = fpp[(i + 1) % 2]
                cdg = work.tile([128, 25, 128], BF16, name=f"cdg{i}",
                                tag="cdg", bufs=2)
                nc.sync.dma_start(out=cdg, in_=cdiag_d[i])
                xn = None
                if i < NHEADS - 1:
                    half = spc * N // 2
                    xnL = work.tile([CH, half], BF16, name=f"xnL{i}",
                                    tag="xnL", bufs=1)
                    nc.sync.dma_start(out=xnL, in_=xh_d[i + 1, 0:CH, 0:half])
                    xnR = work.tile([CH, half], BF16, name=f"xnR{i}",
                                    tag="xnR", bufs=1)
                    nc.sync.dma_start(out=xnR,
                                      in_=xh_d[i + 1, 0:CH, half:2 * half])
                    xn = (xnL, xnR)

                # ================= phase A =================
                kf = [None] * (spc // 4)
                qf = [None] * nhb
                vts = [None] * npair
                for hb in range(nhb):
                    qp3 = qpad[hb].rearrange("p (g r c) -> p g r c", g=2, c=18)
                    for gg in range(2):
                        g = 2 * hb + gg
                        kqp = psA.tile([128, 512], F32, name=f"kqp{g}",
                                       tag="A")
                        for j in range(4):
                            nc.tensor.matmul(
                                kqp[32 * j:32 * j + 32, 0:N],
                                wk_sb[i],
                                fsrc[:, (4 * g + j) * N:(4 * g + j + 1) * N],
                                start=True, stop=True,
                                tile_position=(0, 32 * j),
                            )
                            nc.tensor.matmul(
                                kqp[32 * j:32 * j + 32, 196:196 + N],
                                wq_sb[i],
                                fsrc[:, (4 * g + j) * N:(4 * g + j + 1) * N],
                                start=True, stop=True,
                                tile_position=(0, 32 * j),
                            )
                        t = persist.tile([128, N], BF16, name=f"kf{i}_{g}",
                                         tag=f"kf{g}")
                        nc.vector.tensor_copy(t, kqp[:, 0:N])
                        kf[g] = t
                        nc.vector.tensor_copy(
                            qp3[:, gg, 2:16, 2:16],
                            kqp[:, 196:196 + N].rearrange(
                                "p (r c) -> p r c", c=RES),
                        )
                    dq = psA.tile([128, 512], F32, name=f"dq{hb}", tag="A")
                    for t_i, (dr, dc) in enumerate(TAPS):
                        nc.tensor.matmul(
                            dq[:, 0:392],
                            cdg[:, t_i, :],
                            qp3[:, :, 2 + dr:16 + dr, 2 + dc:16 + dc],
                            start=(t_i == 0), stop=(t_i == len(TAPS) - 1),
                        )
                    g8 = work.tile([128, 392], BF16, name=f"g8{hb}", tag="g8",
                                   bufs=2)
                    nc.scalar.activation(
                        g8, dq[:, 0:392], mybir.ActivationFunctionType.Gelu,
                        bias=dwb_sb[i], scale=1.0,
                    )
                    t = persist.tile([128, 392], BF16, name=f"qf{i}_{hb}",
                                     tag=f"qf{hb}")
                    nc.vector.tensor_add(
                        t.rearrange("p (g r c) -> p g r c", g=2, c=RES),
                        g8.rearrange("p (g r c) -> p g r c", g=2, c=RES),
                        qp3[:, :, 2:16, 2:16],
                    )
                    qf[hb] = t
                    for gg in range(2):
                        g = 2 * hb + gg
                        for pp in range(2):
                            p = 2 * g + pp
                            vt = psA.tile([128, 512], F32, name=f"vt{p}",
                                          tag="A")
                            for u2 in range(2):
                                sl = 2 * p + u2
                                nc.tensor.matmul(
                                    vt[0:128, 65 * u2:65 * u2 + 65],
                                    fsrc[:, sl * N:sl * N + 128], wv_sb[i],
                                    start=True, stop=True,
                                )
                                nc.tensor.matmul(
                                    vt[0:68, 260 + 65 * u2:260 + 65 * u2 + 65],
                                    fsrc[:, sl * N + 128:sl * N + 196],
                                    wv_sb[i],
                                    start=True, stop=True,
                                )
                            t = persist.tile([128, 260], BF16,
                                             name=f"vts{i}_{p}",
                                             tag=f"vts{p % 16}")
                            nc.vector.tensor_copy(t[:, 0:130], vt[:, 0:130])
                            nc.vector.tensor_copy(t[0:68, 130:260],
                                                  vt[0:68, 260:390])
                            vts[p] = t

                # ================= phase B =================
                for p in range(npair):
                    g = p // 2
                    pT = psT.tile([128, 1024], F32, name=f"pT{p}", tag="pT")
                    for u2 in range(2):
                        sl = 2 * p + u2
                        j = sl % 4
                        qs = qf[g // 2].rearrange(
                            "p (g r) -> p g r", g=2)[32 * j:32 * j + 16,
                                                     g % 2, :]
                        nc.tensor.matmul(
                            pT[0:128, 196 * u2:196 * u2 + 196],
                            ident, ab0_sb[i],
                            start=True, stop=False,
                        )
                        nc.tensor.matmul(
                            pT[0:128, 512 + 196 * u2:512 + 196 * u2 + 196],
                            ident[0:68, 0:128], ab1_sb[i],
                            start=True, stop=False,
                        )
                        nc.tensor.matmul(
                            pT[0:128, 196 * u2:196 * u2 + 196],
                            kf[g][32 * j:32 * j + 16, 0:128], qs,
                            start=False, stop=True,
                            tile_position=(32 * j, 0),
                        )
                        nc.tensor.matmul(
                            pT[0:68, 512 + 196 * u2:512 + 196 * u2 + 196],
                            kf[g][32 * j:32 * j + 16, 128:196], qs,
                            start=False, stop=True,
                            tile_position=(32 * j, 0),
                        )
                    eP = work.tile([128, 784], BF16, name=f"eP{p}", tag="eP",
                                   bufs=3)
                    nc.scalar.activation(
                        eP.rearrange("p (t n) -> p t n", t=2),
                        pT.rearrange("p (t n) -> p t n", t=2)[:, :, 0:392],
                        mybir.ActivationFunctionType.Exp,
                    )
                    av = psV.tile([65, 512], F32, name=f"av{p}", tag="av")
                    for u2 in range(2):
                        nc.tensor.matmul(
                            av[:, 196 * u2:196 * u2 + 196],
                            vts[p][0:128, 65 * u2:65 * u2 + 65],
                            eP[0:128, 196 * u2:196 * u2 + 196],
                            start=True, stop=False,
                        )
                        nc.tensor.matmul(
                            av[:, 196 * u2:196 * u2 + 196],
                            vts[p][0:68, 130 + 65 * u2:130 + 65 * u2 + 65],
                            eP[0:68, 392 + 196 * u2:392 + 196 * u2 + 196],
                            start=False, stop=True,
                        )
                    dsb = work.tile([1, 392], F32, name="dsb", tag="dsb",
                                    bufs=3)
                    nc.scalar.copy(dsb, av[64:65, 0:392])
                    rcp = work.tile([1, 392], F32, name="rcp", tag="rcp",
                                    bufs=3)
                    nc.vector.reciprocal_approx_fast(rcp, dsb)
                    bc = work.tile([64, 392], F32, name="bc", tag="bc",
                                   bufs=3)
                    nc.gpsimd.partition_broadcast(bc, rcp[0:1, :])
                    rc = (rcA if i < 2 else rcB)[p][
                        64 * (i % 2):64 * (i % 2) + 64, :]
                    if i < NHEADS - 1:
                        avdiv = work.tile([64, 392], BF16, name="avdiv",
                                          tag="avdiv", bufs=3)
                        nc.vector.tensor_mul(avdiv, av[0:64, 0:392], bc)
                        nc.vector.tensor_scalar_max(rc, avdiv, 0.0)
                        ph = p % (npair // 2)
                        nc.vector.tensor_add(
                            fdst[0:CH, p * 2 * N:(p + 1) * 2 * N],
                            avdiv,
                            xn[p // (npair // 2)][:, ph * 2 * N:
                                                  (ph + 1) * 2 * N])
                    else:
                        nc.vector.scalar_tensor_tensor(
                            rc, av[0:64, 0:392], 0.0, bc,
                            op0=mybir.AluOpType.max,
                            op1=mybir.AluOpType.mult,
                        )
                    if i == NHEADS - 1 and p % 8 == 7:
                        for pp in range(p - 7, p + 1):
                            for m in range(2):
                                op = psA.tile([128, 512], F32,
                                              name=f"op{pp}_{m}", tag="A")
                                nc.tensor.matmul(
                                    op[:, 0:392],
                                    pw0[:, 128 * m:128 * m + 128], rcA[pp],
                                    start=True, stop=False,
                                )
                                nc.tensor.matmul(
                                    op[:, 0:392],
                                    pw1[:, 128 * m:128 * m + 128], rcB[pp],
                                    start=False, stop=True,
                                )
                                ob = outp.tile([128, 392], F32,
                                               name=f"ob{m}", tag=f"ob{m}")
                                if m == 0:
                                    nc.scalar.activation(
                                        ob, op[:, 0:392],
                                        mybir.ActivationFunctionType.Identity,
                                        bias=pb0, scale=1.0,
                                    )
                                else:
                                    nc.vector.tensor_scalar_add(
                                        ob, op[:, 0:392], pb1)
                                nc.sync.dma_start(
                                    out=out_d[
                                        2 * pp:2 * pp + 2,
                                        128 * m:128 * m + 128,
                                        :,
                                    ].rearrange("s o n -> o s n"),
                                    in_=ob.rearrange("o (s n) -> o s n", s=2),
                                )
    nc.finalize()
    return nc


_CACHE = {}


def _get_nc():
    if "nc" not in _CACHE:
        _CACHE["nc"] = build_bass()
    return _CACHE["nc"]


def build_inmaps(inputs):
    host = _prep_host(inputs)
    xp = _pack_x(np.asarray(inputs["x"], np.float32).reshape(BATCH, DIM, N))
    in_maps = []
    for c in range(NCORES):
        m = {"xh": np.ascontiguousarray(xp[c])}
        m.update(host)
        in_maps.append(m)
    return in_maps


def kernel(**inputs) -> np.ndarray:
    from concourse.bass_utils import run_bass_kernel_spmd

    nc = _get_nc()
    in_maps = build_inmaps(inputs)
    res = run_bass_kernel_spmd(nc, in_maps, list(range(NCORES)))
    out = np.concatenate([r["out"] for r in res.results], axis=0)
    return out.reshape(BATCH, DIM, RES, RES).astype(np.float32)
